# revision 25
# baseline (speedup 1.0000x reference)
"""GQA (grouped-query attention) Trainium2 kernel, 8-core SPMD.

Problem: B=4, T=2048, d_model=2048, 32 Q heads, 8 KV heads, d_k=64, causal.
Sharding: core = (batch b, half-of-KV-heads h): 8 cores = 4 batches x 2 halves.
Each core computes its 4 KV heads (16 Q heads) for its batch and the partial
output o_half @ Wo_half (row-parallel Wo); host sums the two halves per batch
and adds bo.

Device-side design (per core), v2 (row-tiled concurrent scores):
  - Scores contract only d_k=64, so two heads' score matmuls are issued as
    K=64 row-tiles on partition ranges 0-63 / 64-127.  Disjoint row-groups
    run CONCURRENTLY in the PE array (16x 32x32 sub-arrays), so a head-PAIR's
    score chunk costs ~512 cycles instead of 2x512 -- the former K=128
    zero-padded formulation wasted half the array.
  - Head slots are host-permuted (Wq columns / bq / Wo rows) so Q-proj PSUM
    chunk m holds head pair (even-parity KV head on rows 0-63, odd-parity on
    rows 64-127), matching the kT row split.  The q buffer U packs both
    heads of a pair in the SAME 512-column range on disjoint partition
    halves (16 KiB/part total, no zero padding, single-eviction per chunk).
  - Scores land transposed (s^T[tk, tq]) in a [128, 2, TQ] PSUM pair tile
    (head A bank 0, head B bank 1); one Exp ACTIVATE covers both heads for
    fully-unmasked chunks (halves the ACT per-instruction overhead, which
    would otherwise outpace the now-2x-faster PE in late tiles).
  - Causal: diagonal chunk di>=1 computes/exps only columns [128di, 512);
    pT's masked region is memset 0; the 128-wide triangle block is
    multiplied by a precomputed mask.  PV matmuls also skip the
    fully-masked columns of diagonal chunks.
  - V carries a ones column so PV also accumulates softmax denominators
    (row 64); per head one DVE reciprocal + GpSimd broadcast + fused
    divide-evict into the oT tile consumed by the O-projection.
  - Software pipelining: a global (pair, chunk) task stream per tile with
    PV trailing the scores by 2 slots ACROSS pair boundaries (the in-order
    PE queue never head-of-line blocks on a pair's tail exps), and a
    per-tile filler schedule paces PE-dense work between attention chunks
    so the scalar engine's exp stream (0.83 ns/col + ~274 ns/instr fixed,
    ~35.7M elements) stays hidden behind PE work:
      startup: K-phase (both kT chunks, 8 PSUM accumulators, paced against
        the xT DMA stream) + V tail 0 + Q-proj(0) slots 0-3;
      tile0: Q-proj(0) slots 4-7 + V tail 1 + Q-proj(1);
      tile1: Q-proj(2) + V tails 2,3;  tile2: Q-proj(3) + O-proj(0);
      tile3: O-proj(1) + O-proj(2);  epilogue: O-proj(3) (alternating
      PSUM pools so back-to-back groups don't serialize).
  - The softmax divide chain is latency-decoupled: raw f32 denominator +
    bf16 numerator evictions free the o65 PSUM immediately; the final
    multiply is deferred 2 tasks so the in-order DVE queue never waits on
    the ~1.3us GpSimd partition_broadcast.
  - Wk/Wv are DMA-hosted in two O-proj tile buffers (dead until tiles 1/2
    rotate them back); xT stays resident in bf16.  DMAs are issued as ~20
    large grouped transfers (descriptor issue costs ~0.6us each on the
    sync queue).
"""

import numpy as np
import ml_dtypes
from contextlib import ExitStack

B, T, D = 4, 2048, 2048
NKV, NREP, DK = 8, 4, 64
HALF_KV = 4                  # kv heads per core
NQH = HALF_KV * NREP         # 16 q heads per core
NPAIR = NQH // 2             # 8 head pairs per core
QD = NQH * DK                # 1024 q dims per core
KVD = HALF_KV * DK           # 256 kv dims per core
NCORES = 8
CD = D // 128                # 16 contraction chunks over d_model
CT = T // 128                # 16 token chunks of 128
TQ = 512                     # query tile width
NTQ = T // TQ                # 4 query tiles
SCALE = 1.0 / np.sqrt(DK)

BF16 = ml_dtypes.bfloat16

# head slot permutation: slot 2m holds an even-parity-KV head (kT rows 0-63),
# slot 2m+1 an odd-parity one (rows 64-127).
SLOT_A = [0, 1, 2, 3, 8, 9, 10, 11]      # kv 0 or 2 -> rows 0-63
SLOT_B = [4, 5, 6, 7, 12, 13, 14, 15]    # kv 1 or 3 -> rows 64-127
HEAD_OF_SLOT = []
for _m in range(8):
    HEAD_OF_SLOT += [SLOT_A[_m], SLOT_B[_m]]
KV_OF_SLOT = [h // NREP for h in HEAD_OF_SLOT]
# q-dim permutation (within this core's 1024 q dims)
QPERM = np.concatenate([np.arange(h * DK, (h + 1) * DK) for h in HEAD_OF_SLOT])

_cache = {}


def _body(ctx, tc, aps):
    import concourse.mybir as mybir
    from concourse.bass import ts, ds

    nc = tc.nc
    f32 = mybir.dt.float32
    bf16 = mybir.dt.bfloat16
    xT, Wq, bqv, Wk, bkv, Wv, bv, Wo, out = (
        aps["xT"], aps["Wq"], aps["bq"], aps["Wk"], aps["bk"], aps["Wv"],
        aps["bv"], aps["Wo"], aps["out"])

    # ---- pools ----------------------------------------------------------
    rp = ctx.enter_context(tc.tile_pool(name="res", bufs=1))
    op = ctx.enter_context(tc.tile_pool(name="ot", bufs=3))
    ptp = ctx.enter_context(tc.tile_pool(name="pt", bufs=3))
    dvp = ctx.enter_context(tc.tile_pool(name="dv", bufs=2))
    orp = ctx.enter_context(tc.tile_pool(name="or", bufs=3))
    wp = ctx.enter_context(tc.tile_pool(name="wk", bufs=3))
    pp = ctx.enter_context(tc.tile_pool(name="ps", bufs=2, space="PSUM"))
    po = ctx.enter_context(tc.tile_pool(name="po", bufs=2, space="PSUM"))
    pj = ctx.enter_context(tc.tile_pool(name="pj", bufs=2, space="PSUM"))

    # ---- resident tiles -------------------------------------------------
    xT_sb = rp.tile([128, CD, T], bf16, tag="xT")           # 64 KiB/part
    Wq_sb = rp.tile([128, CD, QD], bf16, tag="Wq")          # 32 KiB/part
    Wo_sb = rp.tile([128, QD // 128, D], bf16, tag="Wo")    # 32 KiB/part
    kT_sb = rp.tile([128, KVD // 128, T], bf16, tag="kT")   # 8 KiB/part
    v_sb = rp.tile([128, CT, HALF_KV, DK + 1], bf16, tag="v")
    bq_sb = rp.tile([128, QD // 128], f32, tag="bq")
    bk_sb = rp.tile([128, KVD // 128], f32, tag="bk")
    bv_sb = rp.tile([1, KVD], bf16, tag="bv")
    ones_b = rp.tile([1, 128], bf16, tag="ones_b")
    # causal triangle for the partially-masked 128-col block of a diagonal
    # chunk: tri[p, t] = (t >= p)
    tri = rp.tile([128, 128], bf16, tag="tri")
    # U holds the two RESIDENT query buffers (tile j uses U[:, j%2]): pair m
    # of a buffer is a [128, 512] slice: rows 0-63 hold head
    # HEAD_OF_SLOT[2m]'s q^T, rows 64-127 head HEAD_OF_SLOT[2m+1]'s.  The
    # K=64 row-tiled score matmuls read only their half, so no zero padding
    # is needed.
    U = rp.tile([128, 2, NPAIR * TQ], bf16, tag="qt")       # 16 KiB/part
    # Wk/Wv DMA-hosted in two O-proj tile buffers (8 KiB each, dead during
    # startup).  Wk chunk c at flat cols [256c, 256c+256); same for Wv.
    wk_host = op.tile([128, QD // 128, TQ], bf16, tag="oT", name="wk_host")
    wv_host = op.tile([128, QD // 128, TQ], bf16, tag="oT", name="wv_host")
    wk_flat = wk_host.rearrange("p a b -> p (a b)")
    wv_flat = wv_host.rearrange("p a b -> p (a b)")

    # ---- DMA order: per-chunk xT+Wk+Wv first (feeds the c-outer projection
    # waves immediately), small constants mid-stream (needed only at the
    # first evictions), then Wq, then Wo.  Tiny strided DMAs cost ~1-2us
    # fixed latency each, so they must not delay the first compute chunk.
    wk3 = wk_flat[:, :].rearrange("p (c k) -> p c k", c=CD)
    wv3 = wv_flat[:, :].rearrange("p (c k) -> p c k", c=CD)
    for g, (c0, c1) in enumerate([(0, 2), (2, 4), (4, 8), (8, 12), (12, 16)]):
        nc.sync.dma_start(wk3[:, c0:c1, :],
                          Wk[c0 * 128:c1 * 128, :].rearrange(
                              "(c p) k -> p c k", p=128))
        nc.sync.dma_start(wv3[:, c0:c1, :],
                          Wv[c0 * 128:c1 * 128, :].rearrange(
                              "(c p) k -> p c k", p=128))
        nc.sync.dma_start(xT_sb[:, c0:c1, :],
                          xT[c0 * 128:c1 * 128, :].rearrange(
                              "(c p) t -> p c t", p=128))
        if g == 2:
            for cc in range(KVD // 128):
                nc.sync.dma_start(bk_sb[:, cc:cc + 1],
                                  bkv[cc, :].unsqueeze(-1))
            nc.sync.dma_start(bv_sb[:, :], bv[:, :])
    for c in range(QD // 128):
        nc.sync.dma_start(bq_sb[:, c:c + 1], bqv[c, :].unsqueeze(-1))
    nc.sync.dma_start(Wq_sb[:, :, :], Wq[:, :].rearrange("(c p) k -> p c k",
                                                         p=128))
    nc.sync.dma_start(Wo_sb[:, :, :], Wo[:, :].rearrange("(c p) k -> p c k",
                                                         p=128))

    nc.vector.memset(ones_b[:, :], 1.0)
    nc.vector.memset(v_sb[:, :, :, DK:DK + 1], 1.0)
    nc.vector.memset(tri[:, :], 1.0)
    nc.gpsimd.affine_select(
        out=tri[:, :], in_=tri[:, :],
        compare_op=mybir.AluOpType.is_ge, fill=0.0,
        base=0, pattern=[[1, 128]], channel_multiplier=-1)

    # ---- K^T projection, phase A: BOTH kT chunks c-outer, 8 PSUM
    # accumulators spanning all pools, paced exactly against the xT DMA
    # stream (~1.7us PE per chunk vs ~1.6us DMA).
    def k_phase():
        kpp = [pp.tile([128, 2, TQ], f32, tag="ss", name=f"kp0_{n}")
               for n in range(2)]
        kp0 = [kpp[n // 2][:, n % 2, :] for n in range(4)]
        kp1 = [po.tile([128, TQ], f32, tag="o65", name="kp1_0"),
               po.tile([128, TQ], f32, tag="o65", name="kp1_1"),
               pj.tile([128, TQ], f32, tag="pjp", name="kp1_2"),
               pj.tile([128, TQ], f32, tag="pjp", name="kp1_3")]
        for c in range(CD):
            for w in range(2):
                for n in range(4):
                    nc.tensor.matmul((kp0 if w == 0 else kp1)[n][:, :],
                                     wk_flat[:, ds(KVD * c + 128 * w, 128)],
                                     xT_sb[:, c, ts(n, TQ)],
                                     start=(c == 0), stop=(c == CD - 1))
        for w in range(2):
            for n in range(4):
                nc.vector.tensor_scalar_add(kT_sb[:, w, ts(n, TQ)],
                                            (kp0 if w == 0 else kp1)[n][:, :],
                                            bk_sb[:, w:w + 1])

    def v_wave_tail(w):
        # phase B V projection: one mt at a time, 1 outstanding PSUM buffer.
        # Yield only every 4 matmuls so the vp occupies its po buffer for
        # ~1.5 attention chunks instead of ~4 (o65-ring collisions at pair
        # boundaries cost ~12us otherwise).
        for i in range(4):
            mt = 4 * w + i
            vp = po.tile([128, KVD], f32, tag="o65", name=f"vp{w}_{i}")
            for c in range(CD):
                nc.tensor.matmul(vp[:, :], xT_sb[:, c, ts(mt, 128)],
                                 wv_flat[:, ds(KVD * c, KVD)],
                                 start=(c == 0), stop=False)
                if c % 4 == 3:
                    yield
            nc.tensor.matmul(vp[:, :], ones_b[:, :],
                             bv_sb[:, :], start=False, stop=True)
            yield
            nc.vector.tensor_copy(
                v_sb[:, mt, :, 0:DK],
                vp[:, :].rearrange("p (h d) -> p h d", h=HALF_KV))

    # ---- pipelined per-query-tile main loop -----------------------------
    def qproj_group(jj, m):
        # generator: one PE matmul per next() so it can be paced as filler
        ps = pj.tile([128, TQ], f32, tag="pjp", name=f"q{jj}_{m}")
        for c in range(CD):
            nc.tensor.matmul(ps[:, :],
                             Wq_sb[:, c, ts(m, 128)],
                             xT_sb[:, c, ds(jj * TQ, TQ)],
                             start=(c == 0), stop=(c == CD - 1))
            if c < CD - 1:
                yield
        nc.vector.tensor_scalar_add(U[:, jj % 2, ds(TQ * m, TQ)],
                                    ps[:, :], bq_sb[:, m:m + 1])
        yield

    def oproj_group(jj, mt, n, pool=None):
        pool = pool or pj
        ps = pool.tile([128, TQ], f32,
                       tag=("pjp" if pool is pj else "o65"),
                       name=f"o{jj}_{mt}_{n}")
        for c in range(QD // 128):
            nc.tensor.matmul(ps[:, :],
                             oT_tiles[jj][:, c, ts(mt, 128)],
                             Wo_sb[:, c, ts(n, TQ)],
                             start=(c == 0), stop=(c == QD // 128 - 1))
            if c < QD // 128 - 1:
                yield
        os_ = wp.tile([128, TQ], bf16, tag="os", name=f"os{jj}_{mt}_{n}")
        nc.vector.tensor_copy(os_[:, :], ps[:, :])
        nc.sync.dma_start(
            out[ds(jj * TQ + mt * 128, 128), ts(n, TQ)], os_[:, :])
        yield

    def qproj_stream(jj, m0=0):
        for m in range(m0, QD // 128):
            yield from qproj_group(jj, m)

    def oproj_stream(jj, alternate=False):
        for gi, (mt, n) in enumerate(
                (mt, n) for mt in range(TQ // 128) for n in range(D // TQ)):
            pool = po if (alternate and gi % 2) else pj
            yield from oproj_group(jj, mt, n, pool=pool)

    oT_tiles = {}

    # prologue: phase A (K both chunks, DMA-paced), phase B (all V waves
    # per-mt + q^T for tile 0), drained back-to-back
    k_phase()
    # only v chunks 0-3 (tail 0), U slots 0-3 needed before tile0's early
    # pairs; everything else front-fills tiles with exp slack.  v tail w is
    # needed from tile w's first pair onward, so tail(1) fills tile0 and
    # tails(2,3) fill tile1.
    for _ in v_wave_tail(0):
        pass
    for m in range(4):
        for _ in qproj_group(0, m):
            pass

    # filler schedule per tile (generators, run left-to-right)
    fill_plan = {
        0: [lambda: qproj_stream(0, m0=4), lambda: v_wave_tail(1),
            lambda: qproj_stream(1)],
        1: [lambda: qproj_stream(2), lambda: v_wave_tail(2),
            lambda: v_wave_tail(3)],
        2: [lambda: qproj_stream(3), lambda: oproj_stream(0)],
        3: [lambda: oproj_stream(1), lambda: oproj_stream(2)],
    }
    fill_counts = {0: 64 + 20 + 128, 1: 128 + 40, 2: 256, 3: 256}

    for j in range(NTQ):
        oT_sb = op.tile([128, QD // 128, TQ], bf16, tag="oT", name=f"oT{j}")
        oT_tiles[j] = oT_sb
        nkeep = 4 * j + 4

        def chain(gens):
            for g in gens:
                yield from g()
        filler = chain(fill_plan[j])
        n_fill = fill_counts[j]
        n_cks = NPAIR * nkeep
        fill_acc = 0.0
        fill_rate = n_fill / n_cks

        def fill(k):
            for _ in range(k):
                if next(filler, "done") == "done":
                    break

        # global (pair, chunk) task stream: PV trails the scores by 2 slots
        # ACROSS pair boundaries, so the in-order PE queue never waits on
        # a pair's tail exps before starting the next pair's scores.
        tasks = [(m, ck) for m in range(NPAIR) for ck in range(nkeep)]
        o65s = {}
        pTs = {}

        def do_pv(m_, pv_, stop):
            o65e, o65o = o65s[m_]
            kv_e = KV_OF_SLOT[2 * m_]
            kv_o = KV_OF_SLOT[2 * m_ + 1]
            pw = 128 * (pv_ - 4 * j) if pv_ > 4 * j else 0
            nc.tensor.matmul(o65e[:, pw:], v_sb[:, pv_, kv_e, :],
                             pTs[(m_, pv_)][:, 0, pw:],
                             start=(pv_ == 0), stop=stop)
            nc.tensor.matmul(o65o[:, pw:], v_sb[:, pv_, kv_o, :],
                             pTs[(m_, pv_)][:, 1, pw:],
                             start=(pv_ == 0), stop=stop)
            del pTs[(m_, pv_)]
            if stop:
                # o65 complete: cheap raw eviction frees the PSUM fast.
                # The final divide multiply is DEFERRED ~2 tasks so the
                # in-order DVE queue never waits on the slow (~1.3us)
                # GpSimd partition_broadcast.
                for i, o65 in enumerate((o65e, o65o)):
                    rr = dvp.tile([1, TQ], f32, tag="rr")
                    nc.vector.tensor_copy(rr[:, :], o65[64:65, :])
                    orw = orp.tile([64, TQ], bf16, tag="or")
                    nc.vector.tensor_copy(orw[:, :], o65[0:64, :])
                    nc.vector.reciprocal_approx_fast(rr[:, :], rr[:, :])
                    bcs = dvp.tile([64, TQ], f32, tag="bc")
                    nc.gpsimd.partition_broadcast(bcs[:, :], rr[:, :])
                    pending_muls.append((cur_t[0], oT_sb, 64 * i, m_, orw,
                                         bcs))
                del o65s[m_]

        pending_muls = []
        cur_t = [0]

        def drain_muls(now, force=False):
            while pending_muls and (force or now - pending_muls[0][0] >= 2):
                _, osb, r0, mm_, orw, bcs = pending_muls.pop(0)
                nc.vector.tensor_mul(osb[r0:r0 + 64, mm_, :],
                                     orw[:, :], bcs[:, :])

        for t, (m, ck) in enumerate(tasks):
            cur_t[0] = t
            drain_muls(t)
            if ck == 0:
                o65s[m] = (po.tile([65, TQ], f32, tag="o65", name=f"oe{j}_{m}"),
                           po.tile([65, TQ], f32, tag="o65", name=f"oo{j}_{m}"))
            g2 = KV_OF_SLOT[2 * m] // 2
            di = ck - 4 * j
            w = 128 * di if di > 0 else 0
            pT = ptp.tile([128, 2, TQ], bf16, tag="pT")
            ss = pp.tile([128, 2, TQ], f32, tag="ss")
            # row-tiled K=64 score matmuls: the two heads' chunks run
            # concurrently on partition rows 0-63 / 64-127
            nc.tensor.matmul(ss[:, 0, 0:TQ - w],
                             kT_sb[0:64, g2, ts(ck, 128)],
                             U[0:64, j % 2, ds(TQ * m + w, TQ - w)],
                             start=True, stop=True)
            nc.tensor.matmul(ss[:, 1, 0:TQ - w],
                             kT_sb[64:128, g2, ts(ck, 128)],
                             U[64:128, j % 2, ds(TQ * m + w, TQ - w)],
                             start=True, stop=True)
            # one Exp over both heads (their mask widths match)
            if w > 0:
                nc.vector.memset(pT[:, :, 0:w], 0.0)
            nc.scalar.activation(pT[:, :, w:TQ], ss[:, :, 0:TQ - w],
                                 mybir.ActivationFunctionType.Exp,
                                 scale=SCALE)
            if di >= 0:
                # triangle block: columns [128di, 128di+128)
                for i in range(2):
                    nc.vector.tensor_mul(pT[:, i, ds(w, 128)],
                                         pT[:, i, ds(w, 128)],
                                         tri[:, :])
            pTs[(m, ck)] = pT
            if t >= 2:
                pm, pc = tasks[t - 2]
                do_pv(pm, pc, stop=(pc == nkeep - 1))
            fill_acc += fill_rate
            k = int(fill_acc)
            fill_acc -= k
            fill(k)
        for tt in (len(tasks) - 2, len(tasks) - 1):
            pm, pc = tasks[tt]
            do_pv(pm, pc, stop=(pc == nkeep - 1))
        drain_muls(0, force=True)
        fill(n_fill)

    # epilogue: O-projection of the last tile (alternating PSUM pools so
    # back-to-back groups don't serialize on a single buffer's eviction)
    for _ in oproj_stream(NTQ - 1, alternate=True):
        pass


def _build():
    import concourse.mybir as mybir
    import concourse.tile as tile
    from concourse import bacc

    nc = bacc.Bacc("TRN2", target_bir_lowering=False, debug=False,
                   num_devices=NCORES)
    f32, bf16 = mybir.dt.float32, mybir.dt.bfloat16
    aps = {
        "xT": nc.dram_tensor("xT", (D, T), bf16, kind="ExternalInput").ap(),
        "Wq": nc.dram_tensor("Wq", (D, QD), bf16, kind="ExternalInput").ap(),
        "bq": nc.dram_tensor("bq", (QD // 128, 128), f32,
                             kind="ExternalInput").ap(),
        "Wk": nc.dram_tensor("Wk", (D, KVD), bf16, kind="ExternalInput").ap(),
        "bk": nc.dram_tensor("bk", (KVD // 128, 128), f32,
                             kind="ExternalInput").ap(),
        "Wv": nc.dram_tensor("Wv", (D, KVD), bf16, kind="ExternalInput").ap(),
        "bv": nc.dram_tensor("bv", (1, KVD), bf16, kind="ExternalInput").ap(),
        "Wo": nc.dram_tensor("Wo", (QD, D), bf16, kind="ExternalInput").ap(),
        "out": nc.dram_tensor("out", (T, D), bf16, kind="ExternalOutput").ap(),
    }
    with tile.TileContext(nc) as tc:
        with ExitStack() as ctx:
            _body(ctx, tc, aps)
    nc.compile()
    return nc


def _get_nc():
    if "nc" not in _cache:
        _cache["nc"] = _build()
    return _cache["nc"]


def _make_in_maps(x, Wq, bq, Wk, bk, Wv, bv, Wo):
    x = np.asarray(x, np.float32)
    in_maps = []
    for core in range(NCORES):
        b, h = core // 2, core % 2
        Wqh = np.asarray(Wq[:, h * QD:(h + 1) * QD], np.float32)[:, QPERM]
        bqh = np.asarray(bq[h * QD:(h + 1) * QD], np.float32)[QPERM]
        Woh = np.asarray(Wo[h * QD:(h + 1) * QD, :], np.float32)[QPERM, :]
        in_maps.append({
            "xT": np.ascontiguousarray(np.asarray(x[b]).T).astype(BF16),
            "Wq": np.ascontiguousarray(Wqh).astype(BF16),
            "bq": np.ascontiguousarray(bqh.reshape(QD // 128, 128)),
            "Wk": np.asarray(Wk[:, h * KVD:(h + 1) * KVD], np.float32).astype(BF16),
            "bk": np.asarray(bk[h * KVD:(h + 1) * KVD], np.float32).reshape(
                KVD // 128, 128),
            "Wv": np.asarray(Wv[:, h * KVD:(h + 1) * KVD], np.float32).astype(BF16),
            "bv": np.asarray(bv[h * KVD:(h + 1) * KVD], np.float32).reshape(
                1, KVD).astype(BF16),
            "Wo": np.ascontiguousarray(Woh).astype(BF16),
        })
    return in_maps


def kernel(x, Wq, bq, Wk, bk, Wv, bv, Wo, bo, **_):
    from concourse.bass_utils import run_bass_kernel_spmd

    in_maps = _make_in_maps(x, Wq, bq, Wk, bk, Wv, bv, Wo)
    nc = _get_nc()
    res = run_bass_kernel_spmd(nc, in_maps, core_ids=list(range(NCORES)))
    bo = np.asarray(bo, np.float32)
    outs = [np.asarray(res.results[c]["out"], np.float32)
            for c in range(NCORES)]
    return np.stack([outs[2 * b] + outs[2 * b + 1] + bo
                     for b in range(B)], axis=0)


# revision 27
# speedup vs baseline: 1.0022x; 1.0022x over previous
"""GQA (grouped-query attention) Trainium2 kernel, 8-core SPMD.

Problem: B=4, T=2048, d_model=2048, 32 Q heads, 8 KV heads, d_k=64, causal.
Sharding: core = (batch b, half-of-KV-heads h): 8 cores = 4 batches x 2 halves.
Each core computes its 4 KV heads (16 Q heads) for its batch and the partial
output o_half @ Wo_half (row-parallel Wo); host sums the two halves per batch
and adds bo.

Device-side design (per core), v2 (row-tiled concurrent scores):
  - Scores contract only d_k=64, so two heads' score matmuls are issued as
    K=64 row-tiles on partition ranges 0-63 / 64-127.  Disjoint row-groups
    run CONCURRENTLY in the PE array (16x 32x32 sub-arrays), so a head-PAIR's
    score chunk costs ~512 cycles instead of 2x512 -- the former K=128
    zero-padded formulation wasted half the array.
  - Head slots are host-permuted (Wq columns / bq / Wo rows) so Q-proj PSUM
    chunk m holds head pair (even-parity KV head on rows 0-63, odd-parity on
    rows 64-127), matching the kT row split.  The q buffer U packs both
    heads of a pair in the SAME 512-column range on disjoint partition
    halves (16 KiB/part total, no zero padding, single-eviction per chunk).
  - Scores land transposed (s^T[tk, tq]) in a [128, 2, TQ] PSUM pair tile
    (head A bank 0, head B bank 1); one Exp ACTIVATE covers both heads for
    fully-unmasked chunks (halves the ACT per-instruction overhead, which
    would otherwise outpace the now-2x-faster PE in late tiles).
  - Causal: diagonal chunk di>=1 computes/exps only columns [128di, 512);
    pT's masked region is memset 0; the 128-wide triangle block is
    multiplied by a precomputed mask.  PV matmuls also skip the
    fully-masked columns of diagonal chunks.
  - V carries a ones column so PV also accumulates softmax denominators
    (row 64); per head one DVE reciprocal + GpSimd broadcast + fused
    divide-evict into the oT tile consumed by the O-projection.
  - Software pipelining: a global (pair, chunk) task stream per tile with
    PV trailing the scores by 2 slots ACROSS pair boundaries (the in-order
    PE queue never head-of-line blocks on a pair's tail exps), and a
    per-tile filler schedule paces PE-dense work between attention chunks
    so the scalar engine's exp stream (0.83 ns/col + ~274 ns/instr fixed,
    ~35.7M elements) stays hidden behind PE work:
      startup: K-phase (both kT chunks, 8 PSUM accumulators, paced against
        the xT DMA stream) + V tail 0 + Q-proj(0) slots 0-3;
      tile0: Q-proj(0) slots 4-7 + V tail 1 + Q-proj(1);
      tile1: Q-proj(2) + V tails 2,3;  tile2: Q-proj(3) + O-proj(0);
      tile3: O-proj(1) + O-proj(2);  epilogue: O-proj(3) (alternating
      PSUM pools so back-to-back groups don't serialize).
  - The softmax divide chain is latency-decoupled: raw f32 denominator +
    bf16 numerator evictions free the o65 PSUM immediately; the final
    multiply is deferred 2 tasks so the in-order DVE queue never waits on
    the ~1.3us GpSimd partition_broadcast.
  - Wk/Wv are DMA-hosted in two O-proj tile buffers (dead until tiles 1/2
    rotate them back); xT stays resident in bf16.  DMAs are issued as ~20
    large grouped transfers (descriptor issue costs ~0.6us each on the
    sync queue).
"""

import numpy as np
import ml_dtypes
from contextlib import ExitStack

B, T, D = 4, 2048, 2048
NKV, NREP, DK = 8, 4, 64
HALF_KV = 4                  # kv heads per core
NQH = HALF_KV * NREP         # 16 q heads per core
NPAIR = NQH // 2             # 8 head pairs per core
QD = NQH * DK                # 1024 q dims per core
KVD = HALF_KV * DK           # 256 kv dims per core
NCORES = 8
CD = D // 128                # 16 contraction chunks over d_model
CT = T // 128                # 16 token chunks of 128
TQ = 512                     # query tile width
NTQ = T // TQ                # 4 query tiles
SCALE = 1.0 / np.sqrt(DK)

BF16 = ml_dtypes.bfloat16

# head slot permutation: slot 2m holds an even-parity-KV head (kT rows 0-63),
# slot 2m+1 an odd-parity one (rows 64-127).
SLOT_A = [0, 1, 2, 3, 8, 9, 10, 11]      # kv 0 or 2 -> rows 0-63
SLOT_B = [4, 5, 6, 7, 12, 13, 14, 15]    # kv 1 or 3 -> rows 64-127
HEAD_OF_SLOT = []
for _m in range(8):
    HEAD_OF_SLOT += [SLOT_A[_m], SLOT_B[_m]]
KV_OF_SLOT = [h // NREP for h in HEAD_OF_SLOT]
# q-dim permutation (within this core's 1024 q dims)
QPERM = np.concatenate([np.arange(h * DK, (h + 1) * DK) for h in HEAD_OF_SLOT])

_cache = {}


def _body(ctx, tc, aps):
    import concourse.mybir as mybir
    from concourse.bass import ts, ds

    nc = tc.nc
    f32 = mybir.dt.float32
    bf16 = mybir.dt.bfloat16
    xT, Wq, bqv, Wk, bkv, Wv, bv, Wo, out = (
        aps["xT"], aps["Wq"], aps["bq"], aps["Wk"], aps["bk"], aps["Wv"],
        aps["bv"], aps["Wo"], aps["out"])

    # ---- pools ----------------------------------------------------------
    rp = ctx.enter_context(tc.tile_pool(name="res", bufs=1))
    op = ctx.enter_context(tc.tile_pool(name="ot", bufs=3))
    ptp = ctx.enter_context(tc.tile_pool(name="pt", bufs=3))
    dvp = ctx.enter_context(tc.tile_pool(name="dv", bufs=2))
    orp = ctx.enter_context(tc.tile_pool(name="or", bufs=3))
    wp = ctx.enter_context(tc.tile_pool(name="wk", bufs=3))
    pp = ctx.enter_context(tc.tile_pool(name="ps", bufs=2, space="PSUM"))
    po = ctx.enter_context(tc.tile_pool(name="po", bufs=2, space="PSUM"))
    pj = ctx.enter_context(tc.tile_pool(name="pj", bufs=2, space="PSUM"))

    # ---- resident tiles -------------------------------------------------
    xT_sb = rp.tile([128, CD, T], bf16, tag="xT")           # 64 KiB/part
    Wq_sb = rp.tile([128, CD, QD], bf16, tag="Wq")          # 32 KiB/part
    Wo_sb = rp.tile([128, QD // 128, D], bf16, tag="Wo")    # 32 KiB/part
    kT_sb = rp.tile([128, KVD // 128, T], bf16, tag="kT")   # 8 KiB/part
    v_sb = rp.tile([128, CT, HALF_KV, DK + 1], bf16, tag="v")
    bq_sb = rp.tile([128, QD // 128], f32, tag="bq")
    bk_sb = rp.tile([128, KVD // 128], f32, tag="bk")
    bv_sb = rp.tile([1, KVD], bf16, tag="bv")
    ones_b = rp.tile([1, 128], bf16, tag="ones_b")
    # causal triangle for the partially-masked 128-col block of a diagonal
    # chunk: tri[p, t] = (t >= p)
    tri = rp.tile([128, 128], bf16, tag="tri")
    # U holds the two RESIDENT query buffers (tile j uses U[:, j%2]): pair m
    # of a buffer is a [128, 512] slice: rows 0-63 hold head
    # HEAD_OF_SLOT[2m]'s q^T, rows 64-127 head HEAD_OF_SLOT[2m+1]'s.  The
    # K=64 row-tiled score matmuls read only their half, so no zero padding
    # is needed.
    U = rp.tile([128, 2, NPAIR * TQ], bf16, tag="qt")       # 16 KiB/part
    # Wk/Wv DMA-hosted in two O-proj tile buffers (8 KiB each, dead during
    # startup).  Wk chunk c at flat cols [256c, 256c+256); same for Wv.
    wk_host = op.tile([128, QD // 128, TQ], bf16, tag="oT", name="wk_host")
    wv_host = op.tile([128, QD // 128, TQ], bf16, tag="oT", name="wv_host")
    wk_flat = wk_host.rearrange("p a b -> p (a b)")
    wv_flat = wv_host.rearrange("p a b -> p (a b)")

    # ---- DMA order: per-chunk xT+Wk+Wv first (feeds the c-outer projection
    # waves immediately), small constants mid-stream (needed only at the
    # first evictions), then Wq, then Wo.  Tiny strided DMAs cost ~1-2us
    # fixed latency each, so they must not delay the first compute chunk.
    wk3 = wk_flat[:, :].rearrange("p (c k) -> p c k", c=CD)
    wv3 = wv_flat[:, :].rearrange("p (c k) -> p c k", c=CD)
    for g, (c0, c1) in enumerate([(0, 2), (2, 4), (4, 8), (8, 12), (12, 16)]):
        # slow strided transposing transfers go on the GpSimd engine's
        # DMA ring so they don't serialize in front of the big contiguous
        # xT stream on the sync ring during the DMA-paced K-phase
        nc.gpsimd.dma_start(wk3[:, c0:c1, :],
                            Wk[c0 * 128:c1 * 128, :].rearrange(
                                "(c p) k -> p c k", p=128))
        nc.gpsimd.dma_start(wv3[:, c0:c1, :],
                            Wv[c0 * 128:c1 * 128, :].rearrange(
                                "(c p) k -> p c k", p=128))
        nc.sync.dma_start(xT_sb[:, c0:c1, :],
                          xT[c0 * 128:c1 * 128, :].rearrange(
                              "(c p) t -> p c t", p=128))
        if g == 2:
            for cc in range(KVD // 128):
                nc.sync.dma_start(bk_sb[:, cc:cc + 1],
                                  bkv[cc, :].unsqueeze(-1))
            nc.sync.dma_start(bv_sb[:, :], bv[:, :])
    for c in range(QD // 128):
        nc.sync.dma_start(bq_sb[:, c:c + 1], bqv[c, :].unsqueeze(-1))
    nc.sync.dma_start(Wq_sb[:, :, :], Wq[:, :].rearrange("(c p) k -> p c k",
                                                         p=128))
    nc.sync.dma_start(Wo_sb[:, :, :], Wo[:, :].rearrange("(c p) k -> p c k",
                                                         p=128))

    nc.vector.memset(ones_b[:, :], 1.0)
    nc.vector.memset(v_sb[:, :, :, DK:DK + 1], 1.0)
    nc.vector.memset(tri[:, :], 1.0)
    nc.gpsimd.affine_select(
        out=tri[:, :], in_=tri[:, :],
        compare_op=mybir.AluOpType.is_ge, fill=0.0,
        base=0, pattern=[[1, 128]], channel_multiplier=-1)

    # ---- K^T projection, phase A: BOTH kT chunks c-outer, 8 PSUM
    # accumulators spanning all pools, paced exactly against the xT DMA
    # stream (~1.7us PE per chunk vs ~1.6us DMA).
    def k_phase():
        kpp = [pp.tile([128, 2, TQ], f32, tag="ss", name=f"kp0_{n}")
               for n in range(2)]
        kp0 = [kpp[n // 2][:, n % 2, :] for n in range(4)]
        kp1 = [po.tile([128, TQ], f32, tag="o65", name="kp1_0"),
               po.tile([128, TQ], f32, tag="o65", name="kp1_1"),
               pj.tile([128, TQ], f32, tag="pjp", name="kp1_2"),
               pj.tile([128, TQ], f32, tag="pjp", name="kp1_3")]
        for c in range(CD):
            for w in range(2):
                for n in range(4):
                    nc.tensor.matmul((kp0 if w == 0 else kp1)[n][:, :],
                                     wk_flat[:, ds(KVD * c + 128 * w, 128)],
                                     xT_sb[:, c, ts(n, TQ)],
                                     start=(c == 0), stop=(c == CD - 1))
        for w in range(2):
            for n in range(4):
                nc.vector.tensor_scalar_add(kT_sb[:, w, ts(n, TQ)],
                                            (kp0 if w == 0 else kp1)[n][:, :],
                                            bk_sb[:, w:w + 1])

    def v_wave_tail(w):
        # phase B V projection: one mt at a time, 1 outstanding PSUM buffer
        for i in range(4):
            mt = 4 * w + i
            vp = po.tile([128, KVD], f32, tag="o65", name=f"vp{w}_{i}")
            for c in range(CD):
                nc.tensor.matmul(vp[:, :], xT_sb[:, c, ts(mt, 128)],
                                 wv_flat[:, ds(KVD * c, KVD)],
                                 start=(c == 0), stop=False)
                yield
            nc.tensor.matmul(vp[:, :], ones_b[:, :],
                             bv_sb[:, :], start=False, stop=True)
            yield
            nc.vector.tensor_copy(
                v_sb[:, mt, :, 0:DK],
                vp[:, :].rearrange("p (h d) -> p h d", h=HALF_KV))

    # ---- pipelined per-query-tile main loop -----------------------------
    def qproj_group(jj, m):
        # generator: one PE matmul per next() so it can be paced as filler
        ps = pj.tile([128, TQ], f32, tag="pjp", name=f"q{jj}_{m}")
        for c in range(CD):
            nc.tensor.matmul(ps[:, :],
                             Wq_sb[:, c, ts(m, 128)],
                             xT_sb[:, c, ds(jj * TQ, TQ)],
                             start=(c == 0), stop=(c == CD - 1))
            if c < CD - 1:
                yield
        nc.vector.tensor_scalar_add(U[:, jj % 2, ds(TQ * m, TQ)],
                                    ps[:, :], bq_sb[:, m:m + 1])
        yield

    def oproj_group(jj, mt, n, pool=None):
        pool = pool or pj
        ps = pool.tile([128, TQ], f32,
                       tag=("pjp" if pool is pj else "o65"),
                       name=f"o{jj}_{mt}_{n}")
        for c in range(QD // 128):
            nc.tensor.matmul(ps[:, :],
                             oT_tiles[jj][:, c, ts(mt, 128)],
                             Wo_sb[:, c, ts(n, TQ)],
                             start=(c == 0), stop=(c == QD // 128 - 1))
            if c < QD // 128 - 1:
                yield
        os_ = wp.tile([128, TQ], bf16, tag="os", name=f"os{jj}_{mt}_{n}")
        nc.vector.tensor_copy(os_[:, :], ps[:, :])
        nc.sync.dma_start(
            out[ds(jj * TQ + mt * 128, 128), ts(n, TQ)], os_[:, :])
        yield

    def qproj_stream(jj, m0=0):
        for m in range(m0, QD // 128):
            yield from qproj_group(jj, m)

    def oproj_stream(jj, alternate=False):
        for gi, (mt, n) in enumerate(
                (mt, n) for mt in range(TQ // 128) for n in range(D // TQ)):
            pool = po if (alternate and gi % 2) else pj
            yield from oproj_group(jj, mt, n, pool=pool)

    oT_tiles = {}

    # prologue: phase A (K both chunks, DMA-paced), phase B (all V waves
    # per-mt + q^T for tile 0), drained back-to-back
    k_phase()
    # only v chunks 0-3 (tail 0), U slots 0-3 needed before tile0's early
    # pairs; everything else front-fills tiles with exp slack.  v tail w is
    # needed from tile w's first pair onward, so tail(1) fills tile0 and
    # tails(2,3) fill tile1.
    for _ in v_wave_tail(0):
        pass
    for m in range(4):
        for _ in qproj_group(0, m):
            pass

    # filler schedule per tile (generators, run left-to-right)
    fill_plan = {
        0: [lambda: qproj_stream(0, m0=4), lambda: v_wave_tail(1),
            lambda: qproj_stream(1)],
        1: [lambda: qproj_stream(2), lambda: v_wave_tail(2),
            lambda: v_wave_tail(3)],
        2: [lambda: qproj_stream(3), lambda: oproj_stream(0)],
        3: [lambda: oproj_stream(1), lambda: oproj_stream(2)],
    }
    fill_counts = {0: 64 + 68 + 128, 1: 128 + 136, 2: 256, 3: 256}

    for j in range(NTQ):
        oT_sb = op.tile([128, QD // 128, TQ], bf16, tag="oT", name=f"oT{j}")
        oT_tiles[j] = oT_sb
        nkeep = 4 * j + 4

        def chain(gens):
            for g in gens:
                yield from g()
        filler = chain(fill_plan[j])
        n_fill = fill_counts[j]
        n_cks = NPAIR * nkeep
        fill_acc = 0.0
        fill_rate = n_fill / n_cks

        def fill(k):
            for _ in range(k):
                if next(filler, "done") == "done":
                    break

        # global (pair, chunk) task stream: PV trails the scores by 2 slots
        # ACROSS pair boundaries, so the in-order PE queue never waits on
        # a pair's tail exps before starting the next pair's scores.
        tasks = [(m, ck) for m in range(NPAIR) for ck in range(nkeep)]
        o65s = {}
        pTs = {}

        def do_pv(m_, pv_, stop):
            o65e, o65o = o65s[m_]
            kv_e = KV_OF_SLOT[2 * m_]
            kv_o = KV_OF_SLOT[2 * m_ + 1]
            pw = 128 * (pv_ - 4 * j) if pv_ > 4 * j else 0
            nc.tensor.matmul(o65e[:, pw:], v_sb[:, pv_, kv_e, :],
                             pTs[(m_, pv_)][:, 0, pw:],
                             start=(pv_ == 0), stop=stop)
            nc.tensor.matmul(o65o[:, pw:], v_sb[:, pv_, kv_o, :],
                             pTs[(m_, pv_)][:, 1, pw:],
                             start=(pv_ == 0), stop=stop)
            del pTs[(m_, pv_)]
            if stop:
                # o65 complete: cheap raw eviction frees the PSUM fast.
                # The final divide multiply is DEFERRED ~2 tasks so the
                # in-order DVE queue never waits on the slow (~1.3us)
                # GpSimd partition_broadcast.
                for i, o65 in enumerate((o65e, o65o)):
                    rr = dvp.tile([1, TQ], f32, tag="rr")
                    nc.vector.tensor_copy(rr[:, :], o65[64:65, :])
                    orw = orp.tile([64, TQ], bf16, tag="or")
                    nc.vector.tensor_copy(orw[:, :], o65[0:64, :])
                    nc.vector.reciprocal_approx_fast(rr[:, :], rr[:, :])
                    bcs = dvp.tile([64, TQ], f32, tag="bc")
                    nc.gpsimd.partition_broadcast(bcs[:, :], rr[:, :])
                    pending_muls.append((cur_t[0], oT_sb, 64 * i, m_, orw,
                                         bcs))
                del o65s[m_]

        pending_muls = []
        cur_t = [0]

        def drain_muls(now, force=False):
            while pending_muls and (force or now - pending_muls[0][0] >= 2):
                _, osb, r0, mm_, orw, bcs = pending_muls.pop(0)
                nc.vector.tensor_mul(osb[r0:r0 + 64, mm_, :],
                                     orw[:, :], bcs[:, :])

        for t, (m, ck) in enumerate(tasks):
            cur_t[0] = t
            drain_muls(t)
            if ck == 0:
                o65s[m] = (po.tile([65, TQ], f32, tag="o65", name=f"oe{j}_{m}"),
                           po.tile([65, TQ], f32, tag="o65", name=f"oo{j}_{m}"))
            g2 = KV_OF_SLOT[2 * m] // 2
            di = ck - 4 * j
            w = 128 * di if di > 0 else 0
            pT = ptp.tile([128, 2, TQ], bf16, tag="pT")
            ss = pp.tile([128, 2, TQ], f32, tag="ss")
            # row-tiled K=64 score matmuls: the two heads' chunks run
            # concurrently on partition rows 0-63 / 64-127
            nc.tensor.matmul(ss[:, 0, 0:TQ - w],
                             kT_sb[0:64, g2, ts(ck, 128)],
                             U[0:64, j % 2, ds(TQ * m + w, TQ - w)],
                             start=True, stop=True)
            nc.tensor.matmul(ss[:, 1, 0:TQ - w],
                             kT_sb[64:128, g2, ts(ck, 128)],
                             U[64:128, j % 2, ds(TQ * m + w, TQ - w)],
                             start=True, stop=True)
            # one Exp over both heads (their mask widths match)
            if w > 0:
                nc.vector.memset(pT[:, :, 0:w], 0.0)
            nc.scalar.activation(pT[:, :, w:TQ], ss[:, :, 0:TQ - w],
                                 mybir.ActivationFunctionType.Exp,
                                 scale=SCALE)
            if di >= 0:
                # triangle block: columns [128di, 128di+128)
                for i in range(2):
                    nc.vector.tensor_mul(pT[:, i, ds(w, 128)],
                                         pT[:, i, ds(w, 128)],
                                         tri[:, :])
            pTs[(m, ck)] = pT
            if t >= 2:
                pm, pc = tasks[t - 2]
                do_pv(pm, pc, stop=(pc == nkeep - 1))
            fill_acc += fill_rate
            k = int(fill_acc)
            fill_acc -= k
            fill(k)
        for tt in (len(tasks) - 2, len(tasks) - 1):
            pm, pc = tasks[tt]
            do_pv(pm, pc, stop=(pc == nkeep - 1))
        drain_muls(0, force=True)
        fill(n_fill)

    # epilogue: O-projection of the last tile (alternating PSUM pools so
    # back-to-back groups don't serialize on a single buffer's eviction)
    for _ in oproj_stream(NTQ - 1, alternate=True):
        pass


def _build():
    import concourse.mybir as mybir
    import concourse.tile as tile
    from concourse import bacc

    nc = bacc.Bacc("TRN2", target_bir_lowering=False, debug=False,
                   num_devices=NCORES)
    f32, bf16 = mybir.dt.float32, mybir.dt.bfloat16
    aps = {
        "xT": nc.dram_tensor("xT", (D, T), bf16, kind="ExternalInput").ap(),
        "Wq": nc.dram_tensor("Wq", (D, QD), bf16, kind="ExternalInput").ap(),
        "bq": nc.dram_tensor("bq", (QD // 128, 128), f32,
                             kind="ExternalInput").ap(),
        "Wk": nc.dram_tensor("Wk", (D, KVD), bf16, kind="ExternalInput").ap(),
        "bk": nc.dram_tensor("bk", (KVD // 128, 128), f32,
                             kind="ExternalInput").ap(),
        "Wv": nc.dram_tensor("Wv", (D, KVD), bf16, kind="ExternalInput").ap(),
        "bv": nc.dram_tensor("bv", (1, KVD), bf16, kind="ExternalInput").ap(),
        "Wo": nc.dram_tensor("Wo", (QD, D), bf16, kind="ExternalInput").ap(),
        "out": nc.dram_tensor("out", (T, D), bf16, kind="ExternalOutput").ap(),
    }
    with tile.TileContext(nc) as tc:
        with ExitStack() as ctx:
            _body(ctx, tc, aps)
    nc.compile()
    return nc


def _get_nc():
    if "nc" not in _cache:
        _cache["nc"] = _build()
    return _cache["nc"]


def _make_in_maps(x, Wq, bq, Wk, bk, Wv, bv, Wo):
    x = np.asarray(x, np.float32)
    in_maps = []
    for core in range(NCORES):
        b, h = core // 2, core % 2
        Wqh = np.asarray(Wq[:, h * QD:(h + 1) * QD], np.float32)[:, QPERM]
        bqh = np.asarray(bq[h * QD:(h + 1) * QD], np.float32)[QPERM]
        Woh = np.asarray(Wo[h * QD:(h + 1) * QD, :], np.float32)[QPERM, :]
        in_maps.append({
            "xT": np.ascontiguousarray(np.asarray(x[b]).T).astype(BF16),
            "Wq": np.ascontiguousarray(Wqh).astype(BF16),
            "bq": np.ascontiguousarray(bqh.reshape(QD // 128, 128)),
            "Wk": np.asarray(Wk[:, h * KVD:(h + 1) * KVD], np.float32).astype(BF16),
            "bk": np.asarray(bk[h * KVD:(h + 1) * KVD], np.float32).reshape(
                KVD // 128, 128),
            "Wv": np.asarray(Wv[:, h * KVD:(h + 1) * KVD], np.float32).astype(BF16),
            "bv": np.asarray(bv[h * KVD:(h + 1) * KVD], np.float32).reshape(
                1, KVD).astype(BF16),
            "Wo": np.ascontiguousarray(Woh).astype(BF16),
        })
    return in_maps


def kernel(x, Wq, bq, Wk, bk, Wv, bv, Wo, bo, **_):
    from concourse.bass_utils import run_bass_kernel_spmd

    in_maps = _make_in_maps(x, Wq, bq, Wk, bk, Wv, bv, Wo)
    nc = _get_nc()
    res = run_bass_kernel_spmd(nc, in_maps, core_ids=list(range(NCORES)))
    bo = np.asarray(bo, np.float32)
    outs = [np.asarray(res.results[c]["out"], np.float32)
            for c in range(NCORES)]
    return np.stack([outs[2 * b] + outs[2 * b + 1] + bo
                     for b in range(B)], axis=0)
